# revision 21
# baseline (speedup 1.0000x reference)
"""Trainium2 Bass kernel: CapOnlyContrastiveLoss (margin contrastive loss, mean reduction).

reference math (N=8192, D=512, margin=0.2):
    scores[i,j]  = -||im_i - ex_j||        (via gemm identity)
    diag[i]      = -||im_i - s_i||         (only the diagonal of l2_sim(im, s) is used)
    loss         = mean(relu(margin + scores - diag))

Strategy (v2 — host-prepped fp8 DoubleRow):
  * 4x2 core grid over (im rows, ex rows): each of the 8 cores computes a
    [2048, 4096] score tile (N^2/8 elements).
  * All O(N*D) prep happens on HOST: row norms (imsq, exsq, c = margin +
    ||im_i - s_i||), fp8 casts, and the [k, t, o, col] DoubleRow packing of
    the transposed operands.  The device only reads ~3 MB/core.
  * PE: per PSUM bank [128 i, 512 j]: 1 bf16 K=2 matmul adds exsq[j]
    (hi/lo bf16 split), then 2 fp8e4 DoubleRow matmuls (K=256 each)
    accumulate -2*im.ex.  fp8 rounding moves the final mean by ~1.1e-3
    relative (validated on host against the reference).
  * ACT: sq = sqrt(psum + imsq[i]) (bias per-partition), FD=2048 per
    instruction (4 banks), output bf16 to SBUF.
  * DVE: acc_col = sum_j min(sq, c[i])  (tensor_scalar op0=min with fused
    accumulator) on bf16 at high perf mode.
  * Host finishes: loss = sum_cores(EX_R * sum_i c_i - sum(acc)) / N^2.
"""

import numpy as np
import ml_dtypes

import concourse.bacc as bacc
import concourse.bass as bass
import concourse.tile as tile
from concourse import bass_utils, mybir

N, D = 8192, 512
MARGIN = 0.2
P = 128
NJ = 512       # one PSUM bank of fp32
JW = 2048      # epilogue batch along j (4 PSUM banks)
I_GROUPS, J_GROUPS = 4, 2  # 8 cores
IM_R = N // I_GROUPS       # 2048 im rows per core
EX_R = N // J_GROUPS       # 4096 ex rows per core
N_IT = IM_R // P           # 16 i tiles
N_JW = EX_R // JW          # 2 j windows
BANKS = JW // NJ           # 4 banks per window
N_JC = EX_R // NJ          # 8 j chunks of 512
NCOL = N_JW * N_IT + 3     # acc columns (last slot split in 4)

F32 = mybir.dt.float32
BF16 = mybir.dt.bfloat16
FP8 = mybir.dt.float8e4
AF = mybir.ActivationFunctionType
ALU = mybir.AluOpType
DR = mybir.MatmulPerfMode.DoubleRow

E4 = ml_dtypes.float8_e4m3
BF = ml_dtypes.bfloat16

_CACHE = {}


def _emit(tc, nc, imw_d, exw_d, exr_d, imsq_d, cc_d, acc_d):
    from contextlib import ExitStack

    with ExitStack() as ctx:
        singles = ctx.enter_context(tc.tile_pool(name="singles", bufs=1))
        sqp = ctx.enter_context(tc.tile_pool(name="sqp", bufs=3))
        psum = ctx.enter_context(tc.tile_pool(name="psum", bufs=2, space="PSUM"))

        imW = singles.tile([P, N_IT, 2, 2, P], FP8)   # (it, t, o, i_in): -2*im
        exW = singles.tile([P, N_JC, 2, 2, NJ], FP8)  # (jc, t, o, j_in): ex
        exr = singles.tile([1, 2, EX_R], FP8)         # exsq (4,1) DR split
        w2 = singles.tile([1, 2, P], FP8)             # DR weights (4, 1)
        imsq = singles.tile([P, N_IT], F32)
        cc = singles.tile([P, N_IT], F32)
        acc_sb = singles.tile([P, NCOL], F32)
        prime = singles.tile([1, 2, P], BF16)

        nc.vector.memset(w2[:, 0, :], 4.0)
        nc.vector.memset(w2[:, 1, :], 1.0)
        # preamble DMAs spread across 3 DGE queues; slot 0 needs exr and
        # then imW[0] + the whole first j window exW[0:4]
        nc.sync.dma_start(out=exr, in_=exr_d)
        nc.sync.dma_start(out=exW[:, 0:1], in_=exw_d[:, 0:1])
        nc.sync.dma_start(out=exW[:, 2:3], in_=exw_d[:, 2:3])
        nc.sync.dma_start(out=imsq, in_=imsq_d)
        nc.sync.dma_start(out=cc, in_=cc_d)
        nc.gpsimd.dma_start(out=imW[:, 0:2], in_=imw_d[:, 0:2])
        nc.gpsimd.dma_start(out=exW[:, 1:2], in_=exw_d[:, 1:2])
        nc.gpsimd.dma_start(out=exW[:, 3:4], in_=exw_d[:, 3:4])
        for u0, u1 in ((2, 4), (4, 8), (8, 12), (12, 16)):
            nc.gpsimd.dma_start(out=imW[:, u0:u1], in_=imw_d[:, u0:u1])
        nc.scalar.dma_start(out=exW[:, 4:6], in_=exw_d[:, 4:6])
        nc.scalar.dma_start(out=exW[:, 6:8], in_=exw_d[:, 6:8])
        # prime the ACT sqrt table load before the first real activation
        nc.scalar.activation(out=prime, in_=w2, func=AF.Sqrt)

        # HAM warmup: cheap N=128 matmuls keep PE busy (and the clock
        # un-throttled) while the input DMAs land
        warm_ps = psum.tile([P, JW], F32, tag="ps")
        for w in range(56):
            nc.tensor.matmul(warm_ps[:, 0:P], w2, w2,
                             start=True, stop=True, perf_mode=DR)

        def emit_slot(jw, it, ep_splits, cols):
            ps = psum.tile([P, JW], F32, tag="ps")
            # exsq correction first (start=True clears each bank);
            # also fp8 DoubleRow so the PE never switches perf mode
            for b in range(BANKS):
                jsl = slice(jw * JW + b * NJ, jw * JW + (b + 1) * NJ)
                nc.tensor.matmul(ps[:, b * NJ:(b + 1) * NJ],
                                 w2, exr[:, :, jsl],
                                 start=True, stop=False,
                                 perf_mode=DR)
            # fp8 DoubleRow mains: chunk t as consecutive MMs so the
            # stationary operand is reloaded only twice per slot
            for t in range(2):
                for b in range(BANKS):
                    nc.tensor.matmul(ps[:, b * NJ:(b + 1) * NJ],
                                     imW[:, it, t],
                                     exW[:, jw * BANKS + b, t],
                                     start=False, stop=(t == 1),
                                     perf_mode=DR)
            # sq = sqrt(psum + imsq[i])  (ACT, PSUM -> SBUF, bf16 out)
            # acc[:, col] = sum_j min(sq, c)  (DVE, fused accumulate)
            w = JW // ep_splits
            for e, col in zip(range(ep_splits), cols):
                esl = slice(e * w, (e + 1) * w)
                sq = sqp.tile([P, w], BF16, tag=f"sq{e}" if ep_splits > 1 else "sq")
                nc.scalar.activation(out=sq, in_=ps[:, esl], func=AF.Sqrt,
                                     bias=imsq[:, it:it + 1], scale=1.0)
                nc.vector.tensor_scalar(sq, sq, cc[:, it:it + 1], 0.0,
                                        ALU.min, ALU.add,
                                        accum_out=acc_sb[:, col:col + 1])

        ncol = 0
        for jw in range(N_JW):
            for it in range(N_IT):
                last = jw == N_JW - 1 and it == N_IT - 1
                ep = 4 if last else 1
                emit_slot(jw, it, ep, list(range(ncol, ncol + ep)))
                ncol += ep
        assert ncol == NCOL

        nc.sync.dma_start(out=acc_d, in_=acc_sb)


def build_program():
    nc = bacc.Bacc("TRN2", target_bir_lowering=False, debug=False)
    imw_d = nc.dram_tensor("imw", [P, N_IT, 2, 2, P], FP8, kind="ExternalInput").ap()
    exw_d = nc.dram_tensor("exw", [P, N_JC, 2, 2, NJ], FP8, kind="ExternalInput").ap()
    exr_d = nc.dram_tensor("exr", [1, 2, EX_R], FP8, kind="ExternalInput").ap()
    imsq_d = nc.dram_tensor("imsq", [P, N_IT], F32, kind="ExternalInput").ap()
    cc_d = nc.dram_tensor("cc", [P, N_IT], F32, kind="ExternalInput").ap()
    acc_d = nc.dram_tensor("acc", [P, NCOL], F32, kind="ExternalOutput").ap()
    with tile.TileContext(nc) as tc:
        _emit(tc, nc, imw_d, exw_d, exr_d, imsq_d, cc_d, acc_d)
    nc.compile()
    return nc


def get_program():
    if "nc" not in _CACHE:
        _CACHE["nc"] = build_program()
    return _CACHE["nc"]


def _pack_dr(a_t, C):
    """[D, R] fp32 -> [128 k, R//C blk, 2 t, 2 o, C] fp8 with contraction
    index c = 256*t + 128*o + k, column blocks of C."""
    R = a_t.shape[1]
    return np.ascontiguousarray(
        a_t.reshape(2, 2, P, R // C, C).transpose(2, 3, 0, 1, 4).astype(E4))


def make_in_maps(im, s, ex_s):
    im = np.asarray(im, np.float32)
    s = np.asarray(s, np.float32)
    ex = np.asarray(ex_s, np.float32)

    imsq = np.sum(im.astype(np.float64) ** 2, axis=1).astype(np.float32)
    exsq = np.sum(ex.astype(np.float64) ** 2, axis=1).astype(np.float32)
    dd = np.sum((im.astype(np.float64) - s.astype(np.float64)) ** 2, axis=1)
    cc = (MARGIN + np.sqrt(dd)).astype(np.float32)

    hi = (exsq / 4.0).astype(E4)
    lo = (exsq - 4.0 * hi.astype(np.float32)).astype(E4)

    in_maps = []
    cvecs = []
    for c in range(8):
        ig, jg = divmod(c, J_GROUPS)
        i_sl = slice(ig * IM_R, (ig + 1) * IM_R)
        j_sl = slice(jg * EX_R, (jg + 1) * EX_R)
        in_maps.append({
            "imw": _pack_dr(-2.0 * im[i_sl].T, P),
            "exw": _pack_dr(ex[j_sl].T, NJ),
            "exr": np.ascontiguousarray(np.stack([hi[j_sl], lo[j_sl]])[None]),
            "imsq": np.ascontiguousarray(imsq[i_sl].reshape(N_IT, P).T),
            "cc": np.ascontiguousarray(cc[i_sl].reshape(N_IT, P).T),
        })
        cvecs.append(cc[i_sl])
    return in_maps, cvecs


def finish(results, cvecs):
    # per core: sum_ij relu(c_i - sq_ij) = EX_R * sum_i c_i - sum_ij min(sq, c)
    total = 0.0
    for r, cv in zip(results, cvecs):
        total += float(EX_R) * float(np.sum(cv, dtype=np.float64))
        total -= float(np.sum(r["acc"], dtype=np.float64))
    return np.array(total / (float(N) * float(N)), dtype=np.float32)


def kernel(im, s, ex_s):
    nc = get_program()
    in_maps, cvecs = make_in_maps(im, s, ex_s)
    res = bass_utils.run_bass_kernel_spmd(nc, in_maps, core_ids=list(range(8)))
    return finish(res.results, cvecs)


if __name__ == "__main__":
    rng = np.random.default_rng(0)
    im = rng.standard_normal((N, D), dtype=np.float32)
    s = rng.standard_normal((N, D), dtype=np.float32)
    ex = rng.standard_normal((N, D), dtype=np.float32)
    print(kernel(im, s, ex))


# revision 22
# speedup vs baseline: 1.0004x; 1.0004x over previous
"""Trainium2 Bass kernel: CapOnlyContrastiveLoss (margin contrastive loss, mean reduction).

reference math (N=8192, D=512, margin=0.2):
    scores[i,j]  = -||im_i - ex_j||        (via gemm identity)
    diag[i]      = -||im_i - s_i||         (only the diagonal of l2_sim(im, s) is used)
    loss         = mean(relu(margin + scores - diag))

Strategy (v2 — host-prepped fp8 DoubleRow):
  * 4x2 core grid over (im rows, ex rows): each of the 8 cores computes a
    [2048, 4096] score tile (N^2/8 elements).
  * All O(N*D) prep happens on HOST: row norms (imsq, exsq, c = margin +
    ||im_i - s_i||), fp8 casts, and the [k, t, o, col] DoubleRow packing of
    the transposed operands.  The device only reads ~3 MB/core.
  * PE: per PSUM bank [128 i, 512 j]: 1 bf16 K=2 matmul adds exsq[j]
    (hi/lo bf16 split), then 2 fp8e4 DoubleRow matmuls (K=256 each)
    accumulate -2*im.ex.  fp8 rounding moves the final mean by ~1.1e-3
    relative (validated on host against the reference).
  * ACT: sq = sqrt(psum + imsq[i]) (bias per-partition), FD=2048 per
    instruction (4 banks), output bf16 to SBUF.
  * DVE: acc_col = sum_j min(sq, c[i])  (tensor_scalar op0=min with fused
    accumulator) on bf16 at high perf mode.
  * Host finishes: loss = sum_cores(EX_R * sum_i c_i - sum(acc)) / N^2.
"""

import numpy as np
import ml_dtypes

import concourse.bacc as bacc
import concourse.bass as bass
import concourse.tile as tile
from concourse import bass_utils, mybir

N, D = 8192, 512
MARGIN = 0.2
P = 128
NJ = 512       # one PSUM bank of fp32
JW = 2048      # epilogue batch along j (4 PSUM banks)
I_GROUPS, J_GROUPS = 4, 2  # 8 cores
IM_R = N // I_GROUPS       # 2048 im rows per core
EX_R = N // J_GROUPS       # 4096 ex rows per core
N_IT = IM_R // P           # 16 i tiles
N_JW = EX_R // JW          # 2 j windows
BANKS = JW // NJ           # 4 banks per window
N_JC = EX_R // NJ          # 8 j chunks of 512
NCOL = N_JW * N_IT + 3     # acc columns (last slot split in 4)

F32 = mybir.dt.float32
BF16 = mybir.dt.bfloat16
FP8 = mybir.dt.float8e4
AF = mybir.ActivationFunctionType
ALU = mybir.AluOpType
DR = mybir.MatmulPerfMode.DoubleRow

E4 = ml_dtypes.float8_e4m3
BF = ml_dtypes.bfloat16

_CACHE = {}


def _emit(tc, nc, imw_d, exw_d, exr_d, imsq_d, cc_d, acc_d):
    from contextlib import ExitStack

    with ExitStack() as ctx:
        singles = ctx.enter_context(tc.tile_pool(name="singles", bufs=1))
        sqp = ctx.enter_context(tc.tile_pool(name="sqp", bufs=3))
        psum = ctx.enter_context(tc.tile_pool(name="psum", bufs=2, space="PSUM"))

        imW = singles.tile([P, N_IT, 2, 2, P], FP8)   # (it, t, o, i_in): -2*im
        exW = singles.tile([P, N_JC, 2, 2, NJ], FP8)  # (jc, t, o, j_in): ex
        exr = singles.tile([2, EX_R], BF16)           # exsq hi/lo rows
        ones2 = singles.tile([2, P], BF16)
        wfull = singles.tile([P, 2, P], FP8)          # full-K warmup weights
        imsq = singles.tile([P, N_IT], F32)
        cc = singles.tile([P, N_IT], F32)
        acc_sb = singles.tile([P, NCOL], F32)
        prime = singles.tile([P, 2, P], BF16)

        nc.vector.memset(ones2, 1.0)
        nc.vector.memset(wfull, 1.0)
        # preamble DMAs spread across 3 DGE queues; slot 0 needs exr and
        # then imW[0] + the whole first j window exW[0:4]
        nc.sync.dma_start(out=exr, in_=exr_d)
        nc.sync.dma_start(out=exW[:, 0:1], in_=exw_d[:, 0:1])
        nc.sync.dma_start(out=exW[:, 2:3], in_=exw_d[:, 2:3])
        nc.sync.dma_start(out=imsq, in_=imsq_d)
        nc.sync.dma_start(out=cc, in_=cc_d)
        nc.gpsimd.dma_start(out=imW[:, 0:2], in_=imw_d[:, 0:2])
        nc.gpsimd.dma_start(out=exW[:, 1:2], in_=exw_d[:, 1:2])
        nc.gpsimd.dma_start(out=exW[:, 3:4], in_=exw_d[:, 3:4])
        for u0, u1 in ((2, 4), (4, 8), (8, 12), (12, 16)):
            nc.gpsimd.dma_start(out=imW[:, u0:u1], in_=imw_d[:, u0:u1])
        nc.scalar.dma_start(out=exW[:, 4:6], in_=exw_d[:, 4:6])
        nc.scalar.dma_start(out=exW[:, 6:8], in_=exw_d[:, 6:8])
        # prime the ACT sqrt table load before the first real activation
        nc.scalar.activation(out=prime, in_=wfull, func=AF.Sqrt)

        # HAM warmup: cheap N=128 matmuls keep PE busy (and the clock
        # un-throttled) while the input DMAs land
        warm_ps = psum.tile([P, JW], F32, tag="ps")
        for w in range(44):
            nc.tensor.matmul(warm_ps[:, 0:P], wfull, wfull,
                             start=True, stop=True, perf_mode=DR)

        def emit_slot(jw, it, ep_splits, cols):
            ps = psum.tile([P, JW], F32, tag="ps")
            # exsq correction first (start=True clears each bank)
            for b in range(BANKS):
                jsl = slice(jw * JW + b * NJ, jw * JW + (b + 1) * NJ)
                nc.tensor.matmul(ps[:, b * NJ:(b + 1) * NJ],
                                 ones2, exr[:, jsl],
                                 start=True, stop=False)
            # fp8 DoubleRow mains: chunk t as consecutive MMs so the
            # stationary operand is reloaded only twice per slot
            for t in range(2):
                for b in range(BANKS):
                    nc.tensor.matmul(ps[:, b * NJ:(b + 1) * NJ],
                                     imW[:, it, t],
                                     exW[:, jw * BANKS + b, t],
                                     start=False, stop=(t == 1),
                                     perf_mode=DR)
            # sq = sqrt(psum + imsq[i])  (ACT, PSUM -> SBUF, bf16 out)
            # acc[:, col] = sum_j min(sq, c)  (DVE, fused accumulate)
            w = JW // ep_splits
            for e, col in zip(range(ep_splits), cols):
                esl = slice(e * w, (e + 1) * w)
                sq = sqp.tile([P, w], BF16, tag=f"sq{e}" if ep_splits > 1 else "sq")
                nc.scalar.activation(out=sq, in_=ps[:, esl], func=AF.Sqrt,
                                     bias=imsq[:, it:it + 1], scale=1.0)
                nc.vector.tensor_scalar(sq, sq, cc[:, it:it + 1], 0.0,
                                        ALU.min, ALU.add,
                                        accum_out=acc_sb[:, col:col + 1])

        ncol = 0
        for jw in range(N_JW):
            for it in range(N_IT):
                last = jw == N_JW - 1 and it == N_IT - 1
                ep = 4 if last else 1
                emit_slot(jw, it, ep, list(range(ncol, ncol + ep)))
                ncol += ep
        assert ncol == NCOL

        nc.sync.dma_start(out=acc_d, in_=acc_sb)


def build_program():
    nc = bacc.Bacc("TRN2", target_bir_lowering=False, debug=False)
    imw_d = nc.dram_tensor("imw", [P, N_IT, 2, 2, P], FP8, kind="ExternalInput").ap()
    exw_d = nc.dram_tensor("exw", [P, N_JC, 2, 2, NJ], FP8, kind="ExternalInput").ap()
    exr_d = nc.dram_tensor("exr", [2, EX_R], BF16, kind="ExternalInput").ap()
    imsq_d = nc.dram_tensor("imsq", [P, N_IT], F32, kind="ExternalInput").ap()
    cc_d = nc.dram_tensor("cc", [P, N_IT], F32, kind="ExternalInput").ap()
    acc_d = nc.dram_tensor("acc", [P, NCOL], F32, kind="ExternalOutput").ap()
    with tile.TileContext(nc) as tc:
        _emit(tc, nc, imw_d, exw_d, exr_d, imsq_d, cc_d, acc_d)
    nc.compile()
    return nc


def get_program():
    if "nc" not in _CACHE:
        _CACHE["nc"] = build_program()
    return _CACHE["nc"]


def _pack_dr(a_t, C):
    """[D, R] fp32 -> [128 k, R//C blk, 2 t, 2 o, C] fp8 with contraction
    index c = 256*t + 128*o + k, column blocks of C."""
    R = a_t.shape[1]
    return np.ascontiguousarray(
        a_t.reshape(2, 2, P, R // C, C).transpose(2, 3, 0, 1, 4).astype(E4))


def make_in_maps(im, s, ex_s):
    im = np.asarray(im, np.float32)
    s = np.asarray(s, np.float32)
    ex = np.asarray(ex_s, np.float32)

    imsq = np.sum(im.astype(np.float64) ** 2, axis=1).astype(np.float32)
    exsq = np.sum(ex.astype(np.float64) ** 2, axis=1).astype(np.float32)
    dd = np.sum((im.astype(np.float64) - s.astype(np.float64)) ** 2, axis=1)
    cc = (MARGIN + np.sqrt(dd)).astype(np.float32)

    hi = exsq.astype(BF)
    lo = (exsq - hi.astype(np.float32)).astype(BF)

    in_maps = []
    cvecs = []
    for c in range(8):
        ig, jg = divmod(c, J_GROUPS)
        i_sl = slice(ig * IM_R, (ig + 1) * IM_R)
        j_sl = slice(jg * EX_R, (jg + 1) * EX_R)
        in_maps.append({
            "imw": _pack_dr(-2.0 * im[i_sl].T, P),
            "exw": _pack_dr(ex[j_sl].T, NJ),
            "exr": np.ascontiguousarray(np.stack([hi[j_sl], lo[j_sl]])),
            "imsq": np.ascontiguousarray(imsq[i_sl].reshape(N_IT, P).T),
            "cc": np.ascontiguousarray(cc[i_sl].reshape(N_IT, P).T),
        })
        cvecs.append(cc[i_sl])
    return in_maps, cvecs


def finish(results, cvecs):
    # per core: sum_ij relu(c_i - sq_ij) = EX_R * sum_i c_i - sum_ij min(sq, c)
    total = 0.0
    for r, cv in zip(results, cvecs):
        total += float(EX_R) * float(np.sum(cv, dtype=np.float64))
        total -= float(np.sum(r["acc"], dtype=np.float64))
    return np.array(total / (float(N) * float(N)), dtype=np.float32)


def kernel(im, s, ex_s):
    nc = get_program()
    in_maps, cvecs = make_in_maps(im, s, ex_s)
    res = bass_utils.run_bass_kernel_spmd(nc, in_maps, core_ids=list(range(8)))
    return finish(res.results, cvecs)


if __name__ == "__main__":
    rng = np.random.default_rng(0)
    im = rng.standard_normal((N, D), dtype=np.float32)
    s = rng.standard_normal((N, D), dtype=np.float32)
    ex = rng.standard_normal((N, D), dtype=np.float32)
    print(kernel(im, s, ex))
